# revision 24
# baseline (speedup 1.0000x reference)
"""PointPillarScatter3d on 8 TRN2 NeuronCores.

Scatter-to-dense == gather-with-inverse-permutation. The BEV grid
(468*468 = 219024 cells, padded to 8*27776 = 222208) is split into 8
slabs of 27776 cells, one per core. The host routes pillars to their
owner core and buckets them by 128-cell block: the feature row for
the r-th occupied cell of block g sits in slice row r of block g, so
every device load is a contiguous full-bandwidth tile -- no gather
descriptors at all. All float traffic moves in bf16 (tolerance is
2e-2; bf16 rounding is ~4e-3): halves DMA bytes and runs the PE at
1 cycle/row instead of fp32's 4.

Per block the device builds a 0/1 selection matrix
S[r, c] = (c == pos[r, g]) with a single DVE tensor_tensor is_equal
against a free-dim iota (pos[r, g] = cell-within-block of the rank-r
row, 255 for padding), and one matmul T_g^T @ S both scatters rows to
their cells and transposes [row, feature] -> [feature, cell]. Blocks
never hold more than RCAP=112 occupied cells for the target workload
(measured max 106 of 128), so only 112 rank rows are staged; if a
denser input ever shows up, staging falls back to a 128-row program.
"""

import sys
from contextlib import ExitStack

import numpy as np

if "/opt/trn_rl_repo" not in sys.path:
    sys.path.insert(0, "/opt/trn_rl_repo")

NX = 468
NY = 468
NCELLS = NY * NX  # 219024
NF = 128
NP = 150000
NCORES = 8

NBLK = 7  # 128-cell blocks per chunk
CHUNK_CELLS = NBLK * 128  # 896
NCHUNKS = 31
CPC = NCHUNKS * CHUNK_CELLS  # 27776 cells per core; 8*27776 = 222208 >= 219024
NBLKTOT = NCHUNKS * NBLK  # 217 blocks per core
RCAP = 112  # staged rank rows per block (>= max block occupancy, else fallback 128)
NB = 4  # chunks per DMA batch

TRACE = False
LAST_RESULT = None
_NC_CACHE = None


def _batches():
    # small leading batches so compute starts early; bigger ones steady-state
    sizes = [2, 2] + [NB] * ((NCHUNKS - 4) // NB) + [(NCHUNKS - 4) % NB]
    out = []
    c0 = 0
    for nb in sizes:
        if nb:
            out.append((c0, nb))
            c0 += nb
    assert c0 == NCHUNKS
    return out


def _build_bass(reps: int = 1, rcap: int = RCAP):
    from contextlib import nullcontext

    from concourse import bacc, mybir
    import concourse.tile as tile

    nc = bacc.Bacc(None, target_bir_lowering=False, debug=False, num_devices=NCORES)
    feat = nc.declare_dram_parameter(
        "features", [rcap, NBLKTOT, NF], mybir.dt.bfloat16, isOutput=False
    )
    pos_d = nc.declare_dram_parameter(
        "pos", [rcap, 1, NBLKTOT], mybir.dt.bfloat16, isOutput=False
    )
    out = nc.declare_dram_parameter(
        "out", [NF, NCHUNKS, CHUNK_CELLS], mybir.dt.bfloat16, isOutput=True
    )

    with tile.TileContext(nc) as tc, ExitStack() as ctx:
        singles = ctx.enter_context(tc.tile_pool(name="singles", bufs=1))
        pos_pool = ctx.enter_context(tc.tile_pool(name="pos_pool", bufs=2))
        g_pool = ctx.enter_context(tc.tile_pool(name="g_pool", bufs=4))
        s_pool = ctx.enter_context(tc.tile_pool(name="s_pool", bufs=6))
        o_pool = ctx.enter_context(tc.tile_pool(name="o_pool", bufs=4))
        ps_pool = ctx.enter_context(tc.tile_pool(name="ps_pool", bufs=4, space="PSUM"))

        # iota_full[r, j, g] = j; materialized packed so the compare's APs
        # all have innermost stride 1, qualifying for the DVE 2x 16-bit mode
        # (a stride-0 innermost dim disables it). Values < 256: bf16 exact.
        iota_full = singles.tile([rcap, 128, NBLK], mybir.dt.bfloat16)
        nc.gpsimd.iota(
            iota_full[:],
            pattern=[[1, 128], [0, NBLK]],
            base=0,
            channel_multiplier=0,
            allow_small_or_imprecise_dtypes=True,
        )

        rep_loop = tc.For_i(0, reps, 1) if reps > 1 else nullcontext()
        ctx.enter_context(rep_loop)

        pos_sb = pos_pool.tile([rcap, 1, NBLKTOT], mybir.dt.bfloat16)
        nc.sync.dma_start(out=pos_sb[:], in_=pos_d[:])

        for c0, nb in _batches():
            g_b = g_pool.tile([rcap, nb * NBLK, NF], mybir.dt.bfloat16)
            nc.sync.dma_start(
                out=g_b[:], in_=feat[:, c0 * NBLK : (c0 + nb) * NBLK, :]
            )
            o_b = o_pool.tile([NF, nb, CHUNK_CELLS], mybir.dt.bfloat16)
            for j in range(nb):
                ci = c0 + j
                # S stored [rank, cell, block]: in1's innermost dim is then the
                # block axis (stride 1), keeping every AP packed for DVE 2x.
                # Compares must stay on DVE: walrus rejects Pool tensor_tensor
                # (and Pool cannot read PSUM), so Pool only issues the out DMA.
                s_t = s_pool.tile([rcap, 128, NBLK], mybir.dt.bfloat16)
                nc.vector.tensor_tensor(
                    out=s_t[:],
                    in0=iota_full[:],
                    in1=pos_sb[
                        :, 0:1, ci * NBLK : (ci + 1) * NBLK
                    ].to_broadcast([rcap, 128, NBLK]),
                    op=mybir.AluOpType.is_equal,
                )
                ps0 = ps_pool.tile([128, 512], mybir.dt.float32)
                ps1 = ps_pool.tile([128, 512], mybir.dt.float32)
                for b in range(NBLK):
                    dst = (
                        ps0[:, b * 128 : (b + 1) * 128]
                        if b < 4
                        else ps1[:, (b - 4) * 128 : (b - 3) * 128]
                    )
                    nc.tensor.matmul(
                        dst,
                        g_b[:, j * NBLK + b, :],
                        s_t[:, :, b],
                        start=True,
                        stop=True,
                    )
                nc.any.tensor_copy(out=o_b[:, j, 0:512], in_=ps0[:])
                nc.any.tensor_copy(out=o_b[:, j, 512:896], in_=ps1[:, 0:384])
            nc.gpsimd.dma_start(out=out[:, c0 : c0 + nb, :], in_=o_b[:])

    nc.finalize()
    return nc


def _get_nc(reps: int = 1, rcap: int = RCAP):
    global _NC_CACHE
    if _NC_CACHE is None:
        _NC_CACHE = {}
    if (reps, rcap) not in _NC_CACHE:
        _NC_CACHE[(reps, rcap)] = _build_bass(reps, rcap)
    return _NC_CACHE[(reps, rcap)]


def _stage(pillar_features, coords, rcap):
    """Bucket pillars by core / 128-cell block / rank-within-block.

    Returns (in_maps, ok): ok is False if some block holds more than
    rcap pillars (caller then retries with rcap=128, which always fits
    since a 128-cell block has at most 128 distinct occupied cells).
    """
    import ml_dtypes

    feat = np.ascontiguousarray(np.asarray(pillar_features), dtype=np.float32)
    feat_bf = feat.astype(ml_dtypes.bfloat16)
    coords = np.asarray(coords)
    cell = (
        coords[:, 1].astype(np.int64) * (NY * NX)
        + coords[:, 2].astype(np.int64) * NX
        + coords[:, 3].astype(np.int64)
    )
    valid = (coords[:, 0] == 0) & (cell >= 0) & (cell < NCELLS)
    vp = np.flatnonzero(valid)
    cells_v = cell[vp]
    order = np.argsort(cells_v, kind="stable")
    rows_sorted = vp[order]
    cells_sorted = cells_v[order]
    bounds = np.searchsorted(cells_sorted, np.arange(NCORES + 1) * CPC)

    in_maps = []
    for c in range(NCORES):
        lo, hi = bounds[c], bounds[c + 1]
        cnt = hi - lo
        lc = cells_sorted[lo:hi] - c * CPC
        blk = lc >> 7
        starts = np.searchsorted(lc, np.arange(NBLKTOT, dtype=np.int64) << 7)
        rank = np.arange(cnt, dtype=np.int64) - starts[blk]
        if cnt and rank.max() >= rcap:
            return None, False

        staged = np.zeros((rcap, NBLKTOT, NF), dtype=ml_dtypes.bfloat16)
        staged[rank, blk] = feat_bf[rows_sorted[lo:hi]]
        pos = np.full((rcap, 1, NBLKTOT), 255.0, dtype=ml_dtypes.bfloat16)
        pos[rank, 0, blk] = (lc & 127).astype(ml_dtypes.bfloat16)

        in_maps.append({"features": staged, "pos": pos})
    return in_maps, True


def _prepare_in_maps(pillar_features: np.ndarray, coords: np.ndarray) -> list[dict]:
    in_maps, ok = _stage(pillar_features, coords, RCAP)
    if not ok:
        in_maps, ok = _stage(pillar_features, coords, 128)
        assert ok
    return in_maps


def kernel(pillar_features: np.ndarray, coords: np.ndarray) -> np.ndarray:
    global LAST_RESULT
    from concourse.bass_utils import run_bass_kernel_spmd

    in_maps = _prepare_in_maps(pillar_features, coords)
    rcap = in_maps[0]["features"].shape[0]
    res = run_bass_kernel_spmd(
        _get_nc(1, rcap), in_maps, core_ids=list(range(NCORES)), trace=TRACE
    )
    LAST_RESULT = res

    full = np.concatenate(
        [np.asarray(res.results[c]["out"]).reshape(NF, CPC) for c in range(NCORES)],
        axis=1,
    )
    return full[:, :NCELLS].astype(np.float32).reshape(1, NF, NY, NX)


# revision 25
# speedup vs baseline: 2.6583x; 2.6583x over previous
"""PointPillarScatter3d on 8 TRN2 NeuronCores.

Scatter-to-dense == gather-with-inverse-permutation. The BEV grid
(468*468 = 219024 cells, padded to 8*27776 = 222208) is split into 8
slabs of 27776 cells, one per core. The host routes pillars to their
owner core and buckets them by 128-cell block: the feature row for
the r-th occupied cell of block g sits in slice row r of block g, so
every device load is a contiguous full-bandwidth tile -- no gather
descriptors at all. All float traffic moves in bf16 (tolerance is
2e-2; bf16 rounding is ~4e-3): halves DMA bytes and runs the PE at
1 cycle/row instead of fp32's 4.

Per block the device builds a 0/1 selection matrix
S[r, c] = (c == pos[r, g]) with a single DVE tensor_tensor is_equal
against a free-dim iota (pos[r, g] = cell-within-block of the rank-r
row, 255 for padding), and one matmul T_g^T @ S both scatters rows to
their cells and transposes [row, feature] -> [feature, cell]. Blocks
never hold more than RCAP=112 occupied cells for the target workload
(measured max 106 of 128), so only 112 rank rows are staged; if a
denser input ever shows up, staging falls back to a 128-row program.
"""

import sys
from contextlib import ExitStack

import numpy as np

if "/opt/trn_rl_repo" not in sys.path:
    sys.path.insert(0, "/opt/trn_rl_repo")

NX = 468
NY = 468
NCELLS = NY * NX  # 219024
NF = 128
NP = 150000
NCORES = 8

NBLK = 7  # 128-cell blocks per chunk
CHUNK_CELLS = NBLK * 128  # 896
NCHUNKS = 31
CPC = NCHUNKS * CHUNK_CELLS  # 27776 cells per core; 8*27776 = 222208 >= 219024
NBLKTOT = NCHUNKS * NBLK  # 217 blocks per core
RCAP = 112  # staged rank rows per block (>= max block occupancy, else fallback 128)
NB = 4  # chunks per DMA batch

TRACE = False
LAST_RESULT = None
_NC_CACHE = None


def _batches():
    # small leading batches so compute starts early; bigger ones steady-state
    sizes = [2, 2] + [NB] * ((NCHUNKS - 4) // NB) + [(NCHUNKS - 4) % NB]
    out = []
    c0 = 0
    for nb in sizes:
        if nb:
            out.append((c0, nb))
            c0 += nb
    assert c0 == NCHUNKS
    return out


def _build_bass(reps: int = 1, rcap: int = RCAP):
    from contextlib import nullcontext

    from concourse import bacc, mybir
    import concourse.tile as tile

    nc = bacc.Bacc(None, target_bir_lowering=False, debug=False, num_devices=NCORES)
    feat = nc.declare_dram_parameter(
        "features", [rcap, NBLKTOT, NF], mybir.dt.bfloat16, isOutput=False
    )
    pos_d = nc.declare_dram_parameter(
        "pos", [rcap, NBLKTOT, 1], mybir.dt.bfloat16, isOutput=False
    )
    out = nc.declare_dram_parameter(
        "out", [NF, NCHUNKS, CHUNK_CELLS], mybir.dt.bfloat16, isOutput=True
    )

    with tile.TileContext(nc) as tc, ExitStack() as ctx:
        singles = ctx.enter_context(tc.tile_pool(name="singles", bufs=1))
        pos_pool = ctx.enter_context(tc.tile_pool(name="pos_pool", bufs=2))
        g_pool = ctx.enter_context(tc.tile_pool(name="g_pool", bufs=4))
        s_pool = ctx.enter_context(tc.tile_pool(name="s_pool", bufs=6))
        o_pool = ctx.enter_context(tc.tile_pool(name="o_pool", bufs=4))
        ps_pool = ctx.enter_context(tc.tile_pool(name="ps_pool", bufs=4, space="PSUM"))

        # iota_b[r, 0, c] = c; ranks/cell-offsets are < 256 so bf16 is exact
        iota_b = singles.tile([rcap, 1, 128], mybir.dt.bfloat16)
        nc.gpsimd.iota(
            iota_b[:, 0, :],
            pattern=[[1, 128]],
            base=0,
            channel_multiplier=0,
            allow_small_or_imprecise_dtypes=True,
        )

        rep_loop = tc.For_i(0, reps, 1) if reps > 1 else nullcontext()
        ctx.enter_context(rep_loop)

        pos_sb = pos_pool.tile([rcap, NBLKTOT, 1], mybir.dt.bfloat16)
        nc.sync.dma_start(out=pos_sb[:], in_=pos_d[:])

        for c0, nb in _batches():
            g_b = g_pool.tile([rcap, nb * NBLK, NF], mybir.dt.bfloat16)
            nc.sync.dma_start(
                out=g_b[:], in_=feat[:, c0 * NBLK : (c0 + nb) * NBLK, :]
            )
            o_b = o_pool.tile([NF, nb, CHUNK_CELLS], mybir.dt.bfloat16)
            for j in range(nb):
                ci = c0 + j
                s_t = s_pool.tile([rcap, NBLK, 128], mybir.dt.bfloat16)
                # compares must stay on DVE: walrus rejects Pool tensor_tensor
                # (and Pool cannot read PSUM), so Pool only issues the out DMA
                nc.vector.tensor_tensor(
                    out=s_t[:],
                    in0=iota_b[:, 0:1, :].to_broadcast([rcap, NBLK, 128]),
                    in1=pos_sb[:, ci * NBLK : (ci + 1) * NBLK, :].to_broadcast(
                        [rcap, NBLK, 128]
                    ),
                    op=mybir.AluOpType.is_equal,
                )
                ps0 = ps_pool.tile([128, 512], mybir.dt.float32)
                ps1 = ps_pool.tile([128, 512], mybir.dt.float32)
                for b in range(NBLK):
                    dst = (
                        ps0[:, b * 128 : (b + 1) * 128]
                        if b < 4
                        else ps1[:, (b - 4) * 128 : (b - 3) * 128]
                    )
                    nc.tensor.matmul(
                        dst,
                        g_b[:, j * NBLK + b, :],
                        s_t[:, b, :],
                        start=True,
                        stop=True,
                    )
                nc.any.tensor_copy(out=o_b[:, j, 0:512], in_=ps0[:])
                nc.any.tensor_copy(out=o_b[:, j, 512:896], in_=ps1[:, 0:384])
            nc.gpsimd.dma_start(out=out[:, c0 : c0 + nb, :], in_=o_b[:])

    nc.finalize()
    return nc


def _get_nc(reps: int = 1, rcap: int = RCAP):
    global _NC_CACHE
    if _NC_CACHE is None:
        _NC_CACHE = {}
    if (reps, rcap) not in _NC_CACHE:
        _NC_CACHE[(reps, rcap)] = _build_bass(reps, rcap)
    return _NC_CACHE[(reps, rcap)]


def _stage(pillar_features, coords, rcap):
    """Bucket pillars by core / 128-cell block / rank-within-block.

    Returns (in_maps, ok): ok is False if some block holds more than
    rcap pillars (caller then retries with rcap=128, which always fits
    since a 128-cell block has at most 128 distinct occupied cells).
    """
    import ml_dtypes

    feat = np.ascontiguousarray(np.asarray(pillar_features), dtype=np.float32)
    feat_bf = feat.astype(ml_dtypes.bfloat16)
    coords = np.asarray(coords)
    cell = (
        coords[:, 1].astype(np.int64) * (NY * NX)
        + coords[:, 2].astype(np.int64) * NX
        + coords[:, 3].astype(np.int64)
    )
    valid = (coords[:, 0] == 0) & (cell >= 0) & (cell < NCELLS)
    vp = np.flatnonzero(valid)
    cells_v = cell[vp]
    order = np.argsort(cells_v, kind="stable")
    rows_sorted = vp[order]
    cells_sorted = cells_v[order]
    bounds = np.searchsorted(cells_sorted, np.arange(NCORES + 1) * CPC)

    in_maps = []
    for c in range(NCORES):
        lo, hi = bounds[c], bounds[c + 1]
        cnt = hi - lo
        lc = cells_sorted[lo:hi] - c * CPC
        blk = lc >> 7
        starts = np.searchsorted(lc, np.arange(NBLKTOT, dtype=np.int64) << 7)
        rank = np.arange(cnt, dtype=np.int64) - starts[blk]
        if cnt and rank.max() >= rcap:
            return None, False

        staged = np.zeros((rcap, NBLKTOT, NF), dtype=ml_dtypes.bfloat16)
        staged[rank, blk] = feat_bf[rows_sorted[lo:hi]]
        pos = np.full((rcap, NBLKTOT, 1), 255.0, dtype=ml_dtypes.bfloat16)
        pos[rank, blk, 0] = (lc & 127).astype(ml_dtypes.bfloat16)

        in_maps.append({"features": staged, "pos": pos})
    return in_maps, True


def _prepare_in_maps(pillar_features: np.ndarray, coords: np.ndarray) -> list[dict]:
    in_maps, ok = _stage(pillar_features, coords, RCAP)
    if not ok:
        in_maps, ok = _stage(pillar_features, coords, 128)
        assert ok
    return in_maps


def kernel(pillar_features: np.ndarray, coords: np.ndarray) -> np.ndarray:
    global LAST_RESULT
    from concourse.bass_utils import run_bass_kernel_spmd

    in_maps = _prepare_in_maps(pillar_features, coords)
    rcap = in_maps[0]["features"].shape[0]
    res = run_bass_kernel_spmd(
        _get_nc(1, rcap), in_maps, core_ids=list(range(NCORES)), trace=TRACE
    )
    LAST_RESULT = res

    full = np.concatenate(
        [np.asarray(res.results[c]["out"]).reshape(NF, CPC) for c in range(NCORES)],
        axis=1,
    )
    return full[:, :NCELLS].astype(np.float32).reshape(1, NF, NY, NX)
